# revision 15
# baseline (speedup 1.0000x reference)
"""GCNConv kernel for 8x Trainium2 NeuronCores (Bass/Tile).

Reference computation:
    h = x @ W + b                  # [N, 256] @ [256, 128] -> [N, 128]
    out[i] = sum_{e: dst[e]=i} val[e] * h[src[e]]

Strategy (per core; SPMD - one program, per-core data):
  - dst nodes sharded 12500/core (output rows).  Edges partitioned by dst.
  - Phase 1: every core computes the full h (fp16) into 4 per-window DRAM
    tensors via PE matmuls (host passes x transposed + fp16), so phase-2
    gathers of window w can start as soon as window w is projected.
  - Bias handled as a rank-1 matmul per dst tile: out_tile += deg_w (x) bias
    (deg_w = per-dst sum of incident edge vals, host-precomputed).
  - Phase 2: per-edge h rows pulled on-chip with dma_gather (int16 indices,
    4 source windows of <=25088 rows), SWDGE queues cycled.  Per 128-edge
    chunk, a host-built staircase matrix B [128e x 32seg] fp16 (carrying
    val) is the stationary matmul operand -> PSUM partial segment sums.  A
    second host-built one-hot S2 [128seg x 128dst] matmul accumulates
    segments into per-dst-tile PSUM, which is written out fp32.
  - Chunks are packed at (group, window) granularity (crossing dst-tile
    boundaries) to minimize gather padding.
All data-dependent structure is padded to the max across cores so the same
program serves all 8 cores.
"""

import os
import sys

for _p in ("/opt/trn_rl_repo",):
    if _p not in sys.path:
        sys.path.insert(0, _p)

import numpy as np

P = 128
MSEG = 32            # segment slots per 128-edge chunk
TPG = 4              # dst tiles (of 128 dst) per processing group
N_CORES = 8
RB = 512             # projection row-batch


def _ceil_to(a, m):
    return -(-a // m) * m


class Plan:
    """Static (core-invariant) program structure + per-core data arrays."""


def build_plan(x, edge_src, edge_dst, edge_vals, weight, bias):
    N, IN_F = x.shape
    OUT_F = weight.shape[1]
    assert N % N_CORES == 0
    ndst = N // N_CORES                    # dst nodes per core
    ndst_pad = _ceil_to(ndst, P)
    ntile = ndst_pad // P                  # dst tiles per core
    ngrp = -(-ntile // TPG)
    hrows = _ceil_to(N, P)
    blkr = 25088                           # window rows (mult of 512, <32767)
    nblk = -(-hrows // blkr)
    wrows = [min(blkr, hrows - w * blkr) for w in range(nblk)]

    pl = Plan()
    pl.N, pl.IN_F, pl.OUT_F = N, IN_F, OUT_F
    pl.ndst, pl.ndst_pad, pl.ntile, pl.ngrp = ndst, ndst_pad, ntile, ngrp
    pl.hrows, pl.nblk, pl.blkr, pl.wrows = hrows, nblk, blkr, wrows
    pl.kc = IN_F // P
    pl.grp_tiles = [list(range(g * TPG, min((g + 1) * TPG, ntile)))
                    for g in range(ngrp)]

    # --- dense inputs ---
    import ml_dtypes
    f8 = ml_dtypes.float8_e4m3
    xf8 = os.environ.get("K_XF8", "0") == "1"
    xdt = f8 if xf8 else np.float16
    pl.xf8 = xf8
    xT = np.zeros((pl.kc, P, hrows), xdt)
    xT[:, :, :N] = np.ascontiguousarray(x.astype(np.float32).T).reshape(
        pl.kc, P, N).astype(xdt)
    pl.xT = xT
    pl.W = np.ascontiguousarray(
        weight.astype(np.float32).reshape(pl.kc, P, OUT_F).transpose(
            1, 0, 2)).astype(xdt)
    pl.bvec = np.ascontiguousarray(bias.astype(np.float16)[None, :])

    deg_w = np.bincount(edge_dst, weights=edge_vals.astype(np.float64),
                        minlength=N).astype(np.float32)
    degw = np.zeros((N_CORES, 1, ndst_pad), np.float16)
    degw[:, 0, :ndst] = deg_w.reshape(N_CORES, ndst).astype(np.float16)
    pl.degw = degw

    # --- edges, sorted for (core, grp, blk) streams ---
    src = edge_src.astype(np.int64)
    dst = edge_dst.astype(np.int64)
    val = edge_vals.astype(np.float32)
    E = len(src)
    core = dst // ndst
    dl = dst % ndst
    grp = (dl // P) // TPG
    blk = src // blkr
    order = np.lexsort((src, dl, blk, grp, core))
    src, dst, val = src[order], dst[order], val[order]
    core, dl, grp, blk = core[order], dl[order], grp[order], blk[order]

    nbuck = ngrp * nblk
    bucket = (core * ngrp + grp) * nblk + blk
    bcount = np.bincount(bucket, minlength=N_CORES * nbuck)
    bstart = np.concatenate([[0], np.cumsum(bcount)[:-1]])
    slot_in_b = np.arange(E) - bstart[bucket]

    run_change = np.ones(E, bool)
    run_change[1:] = (bucket[1:] != bucket[:-1]) | (dl[1:] != dl[:-1])

    # fast path: chunks are fixed 128-slot windows of each bucket stream
    def compute_cd(slot_in_b):
        chunk_loc = slot_in_b // P
        piece_flag = run_change | (slot_in_b % P == 0)
        piece_id = np.cumsum(piece_flag) - 1
        cs = np.where(slot_in_b % P == 0, piece_id, -1)
        first = np.maximum.accumulate(cs)
        d = piece_id - first
        return chunk_loc, piece_flag, d

    chunk_loc, piece_flag, d = compute_cd(slot_in_b)

    if d.max() >= MSEG:
        # rare: some 128-slot window has >MSEG runs; redo those buckets with
        # a greedy that closes chunks early (slot padding inside the bucket)
        bad = np.unique(bucket[d >= MSEG])
        for bb in bad:
            lo, n = bstart[bb], bcount[bb]
            hi = lo + n
            rc = run_change[lo:hi]
            starts = np.nonzero(rc)[0]
            lens = np.diff(np.append(starts, n))
            pos = np.empty(n, np.int64)
            c, s, dd = 0, 0, 0
            for st, ln in zip(starts, lens):
                rem, off = ln, 0
                while rem > 0:
                    if s == P or dd == MSEG:
                        c += 1
                        s, dd = 0, 0
                    take = min(P - s, rem)
                    pos[st + off:st + off + take] = c * P + s + np.arange(take)
                    s += take
                    dd += 1
                    rem -= take
                    off += take
            slot_in_b[lo:hi] = pos
        chunk_loc, piece_flag, d = compute_cd(slot_in_b)
        assert d.max() < MSEG

    # per-bucket chunk counts -> static maxima
    cc_b = np.zeros(N_CORES * nbuck, np.int64)
    has = bcount > 0
    last_idx = bstart + bcount - 1
    cc_b[has] = chunk_loc[last_idx[has]] + 1
    cc_cb = cc_b.reshape(N_CORES, ngrp, nblk)
    CH = cc_cb.max(axis=0)                 # [ngrp, nblk] static chunk counts
    CC_g = CH.sum(axis=1)                  # per-group chunks
    CC = int(CC_g.sum())
    TOT = CC * P

    choff = np.zeros((ngrp, nblk), np.int64)
    c_lo = np.zeros(ngrp + 1, np.int64)
    off = 0
    for g in range(ngrp):
        c_lo[g] = off
        for b in range(nblk):
            choff[g, b] = off
            off += int(CH[g, b])
    c_lo[ngrp] = off
    assert off == CC

    chunk_glob = choff[grp, blk] + chunk_loc
    slot_glob = choff[grp, blk] * P + slot_in_b

    # idx array (int16 window-local row), wrapped per 16, replicated x8
    idx_flat = np.zeros((N_CORES, TOT), np.int16)
    idx_flat[core, slot_glob] = (src - blk * blkr).astype(np.int16)
    IDX = np.ascontiguousarray(
        idx_flat.reshape(N_CORES, TOT // 16, 16).transpose(0, 2, 1))
    IDX = np.tile(IDX, (1, 8, 1))          # [N_CORES, 128, TOT // 16]
    pl.IDX = IDX

    # B staircase (vals)
    Bf = np.zeros((N_CORES, P, CC * MSEG), np.float16)
    Bf[core, slot_glob % P, chunk_glob * MSEG + d] = val.astype(np.float16)
    pl.Bf = Bf

    # segments (pieces) -> L2 one-hot S2
    pidx = np.nonzero(piece_flag)[0]
    p_core = core[pidx]
    p_g = grp[pidx]
    p_crel = chunk_glob[pidx] - c_lo[p_g]
    p_j = p_crel // 4
    p_band = p_crel % 4
    p_d = d[pidx]
    p_dl = dl[pidx]
    p_tile = p_dl // P                     # 0..ntile-1

    # program-static mm list: union over cores of (g, j, tile)
    key = (p_g * 100000 + p_j) * 1000 + p_tile
    ukey = np.unique(key)
    mm_g = ukey // 100000000
    mm_j = (ukey // 1000) % 100000
    mm_t = ukey % 1000
    NMM = len(ukey)
    mm_of_key = {int(k): i for i, k in enumerate(ukey)}
    p_mm = np.searchsorted(ukey, key)

    s2f8 = os.environ.get("K_S2F8", "0") == "1"
    S2f = np.zeros((N_CORES, P, NMM * P),
                   f8 if s2f8 else np.float16)
    S2f[p_core, p_band * MSEG + p_d, p_mm * P + (p_dl % P)] = 1.0
    pl.S2f = S2f
    pl.s2f8 = s2f8

    # per-(g,tile) mm bookkeeping for start/stop flags
    mm_stop = np.zeros(NMM, bool)
    seen = {}
    for i in range(NMM):
        seen[(int(mm_g[i]), int(mm_t[i]))] = i
    for (_, _), i in seen.items():
        mm_stop[i] = True
    # first mm index per group (columns of S2 are mm-ordered = (g, j, t))
    mm0_g = np.searchsorted(mm_g, np.arange(ngrp), side="left")
    nmm_g = np.searchsorted(mm_g, np.arange(ngrp), side="right") - mm0_g

    pl.CC, pl.TOT, pl.NMM = CC, TOT, NMM
    pl.CH, pl.CC_g, pl.choff, pl.c_lo = CH, CC_g, choff, c_lo
    pl.mm_g, pl.mm_j, pl.mm_t, pl.mm_stop = mm_g, mm_j, mm_t, mm_stop
    pl.mm0_g, pl.nmm_g = mm0_g, nmm_g
    return pl


# ---------------------------------------------------------------------------
# Device program
# ---------------------------------------------------------------------------

def build_bass(pl):
    import concourse.bass as bass
    import concourse.mybir as mybir
    import concourse.tile as tile
    from concourse import bacc

    f16 = mybir.dt.float16
    f32 = mybir.dt.float32
    f8 = mybir.dt.float8e4
    i16 = mybir.dt.int16

    NSWQ = int(os.environ.get("K_NSWQ", "4"))
    GMAX = int(os.environ.get("K_GMAX", "8192"))
    nc = bacc.Bacc("TRN2", target_bir_lowering=False, debug=False,
                   num_swdge_queues=NSWQ)

    OF = pl.OUT_F
    xdt = f8 if pl.xf8 else f16
    xT_d = nc.dram_tensor("xt", [pl.kc, P, pl.hrows], xdt,
                          kind="ExternalInput")
    W_d = nc.dram_tensor("w", [P, pl.kc, OF], xdt, kind="ExternalInput")
    b_d = nc.dram_tensor("bvec", [1, OF], f16, kind="ExternalInput")
    dw_d = nc.dram_tensor("degw", [1, pl.ndst_pad], f16,
                          kind="ExternalInput")
    idx_d = nc.dram_tensor("idx", [P, pl.TOT // 16], i16,
                           kind="ExternalInput")
    B_d = nc.dram_tensor("bmat", [P, pl.CC * MSEG], f16,
                         kind="ExternalInput")
    S2_d = nc.dram_tensor("s2", [P, pl.NMM * P],
                          f8 if pl.s2f8 else f16,
                          kind="ExternalInput")
    out_d = nc.dram_tensor("out", [pl.ndst_pad, OF], f32,
                           kind="ExternalOutput")
    h_ds = [nc.dram_tensor(f"hbuf{w}", [pl.wrows[w], OF], f16)
            for w in range(pl.nblk)]

    qctr = [0]

    def next_q():
        q = qctr[0] % NSWQ
        qctr[0] += 1
        return q

    with tile.TileContext(nc) as tc:
        with tc.tile_pool(name="pconst", bufs=1) as pconst:
            W_sb = pconst.tile([P, pl.kc, OF], xdt)
            nc.sync.dma_start(W_sb[:], W_d[:])
            # deg_w / bias padded to K=128 (row 0 live, rest zero) so the
            # rank-1 bias matmul uses the same PE tile config as the S2 mms
            dw2 = pconst.tile([P, pl.ndst_pad], f16)
            nc.gpsimd.memset(dw2[:], 0.0)
            nc.sync.dma_start(dw2[0:1, :], dw_d[:])
            b2 = pconst.tile([P, OF], f16)
            nc.gpsimd.memset(b2[:], 0.0)
            nc.sync.dma_start(b2[0:1, :], b_d[:])

            # ------------- Phase 1: h = x @ W (per window) -------------
            with (
                tc.tile_pool(name="pxt", bufs=3) as pxt,
                tc.tile_pool(name="phs", bufs=3) as phs,
                tc.tile_pool(name="ppsum", bufs=2, space="PSUM") as ppsum,
                tc.tile_pool(name="pidx", bufs=4) as pidx,
                tc.tile_pool(name="pmsg", bufs=8) as pmsg,
                tc.tile_pool(name="pB", bufs=2) as pB,
                tc.tile_pool(name="pS2", bufs=2) as pS2,
                tc.tile_pool(name="pP",
                             bufs=int(max(-(-pl.CC_g // 16))) + 2) as pP,
                tc.tile_pool(name="pout", bufs=2) as pout,
                tc.tile_pool(name="psL1", bufs=2, space="PSUM") as psL1,
                tc.tile_pool(name="psL2", bufs=2, space="PSUM") as psL2,
            ):
                for w in range(pl.nblk):
                    r0g = w * pl.blkr
                    for rb in range(-(-pl.wrows[w] // RB)):
                        r0 = rb * RB
                        nrows = min(RB, pl.wrows[w] - r0)
                        nch = nrows // P
                        xt = pxt.tile([P, pl.kc, RB], xdt, tag="xt")
                        nc.sync.dma_start(
                            xt[:, :, :nrows],
                            xT_d[:, :, r0g + r0:r0g + r0 + nrows].rearrange(
                                "k p c -> p k c"),
                        )
                        ps = ppsum.tile([P, RB], f32, tag="pj")
                        for rc in range(nch):
                            for k in range(pl.kc):
                                nc.tensor.matmul(
                                    ps[:, rc * P:(rc + 1) * P],
                                    lhsT=xt[:, k, rc * P:(rc + 1) * P],
                                    rhs=W_sb[:, k, :],
                                    start=(k == 0),
                                    stop=(k == pl.kc - 1),
                                )
                        hs = phs.tile([P, RB], f16, tag="hs")
                        nc.vector.tensor_copy(hs[:, :nrows], ps[:, :nrows])
                        nc.sync.dma_start(
                            h_ds[w][r0:r0 + nrows, :].rearrange(
                                "(c p) f -> p c f", p=P),
                            hs[:, :nrows].rearrange("p (c f) -> p c f", f=OF),
                        )

                # ------------- Phase 2: gather + L1 + L2 -------------
                for g in range(pl.ngrp):
                    tiles_g = pl.grp_tiles[g]
                    ccg = int(pl.CC_g[g])
                    clo = int(pl.c_lo[g])
                    # group idx tile + gathers per window
                    ixg = pidx.tile([P, ccg * 8], i16, tag="idx")
                    nc.sync.dma_start(
                        ixg[:], idx_d[:, clo * 8:(clo + ccg) * 8])
                    msgs = {}
                    for b in range(pl.nblk):
                        n = int(pl.CH[g, b]) * P
                        if n == 0:
                            continue
                        o = int(pl.choff[g, b]) * P     # global slot offset
                        ol = o - clo * P                # offset within group
                        mt = pmsg.tile([P, n // P, OF], f16, tag="msg")
                        for q0 in range(0, n, GMAX):
                            qn = min(GMAX, n - q0)
                            nc.gpsimd.dma_gather(
                                out_ap=mt[:, q0 // P:(q0 + qn) // P, :],
                                in_ap=h_ds[b][:, :],
                                idxs_ap=ixg[:, (ol + q0) // 16:
                                            (ol + q0 + qn) // 16],
                                num_idxs=qn,
                                num_idxs_reg=qn,
                                elem_size=OF,
                                single_packet=False,
                                queue_num=next_q(),
                            )
                        msgs[b] = mt

                    # B + S2 for the whole group
                    Bt = pB.tile([P, ccg * MSEG], f16, tag="B")
                    nc.sync.dma_start(
                        Bt[:], B_d[:, clo * MSEG:(clo + ccg) * MSEG])
                    mm0 = int(pl.mm0_g[g])
                    nmm = int(pl.nmm_g[g])
                    s2t = pS2.tile([P, max(nmm, 1) * P],
                   f8 if pl.s2f8 else f16, tag="s2")
                    if nmm:
                        nc.sync.dma_start(
                            s2t[:, :nmm * P],
                            S2_d[:, mm0 * P:(mm0 + nmm) * P])

                    # window of each chunk (static)
                    win_of = np.repeat(np.arange(pl.nblk),
                                       pl.CH[g]).astype(np.int64)

                    pts = []
                    n_pg = -(-ccg // 16)
                    for pg in range(n_pg):
                        cc0 = 16 * pg
                        nch = min(16, ccg - cc0)
                        nw2 = -(-nch // 4)
                        ps1 = psL1.tile([P, 16 * MSEG], f32, tag="ps1")
                        for cc2 in range(nch):
                            crel = cc0 + cc2
                            b = int(win_of[crel])
                            ci = clo + crel - int(pl.choff[g, b])
                            band = cc2 % 4
                            w2 = cc2 // 4
                            nc.tensor.matmul(
                                ps1[MSEG * band:MSEG * (band + 1),
                                    w2 * P:(w2 + 1) * P],
                                lhsT=Bt[:, crel * MSEG:(crel + 1) * MSEG],
                                rhs=msgs[b][:, ci, :],
                                start=True,
                                stop=True,
                                tile_position=(0, MSEG * band),
                            )
                        Pt = pP.tile([P, 4, OF], f16, tag="P")
                        nc.vector.tensor_copy(
                            Pt[:, :nw2, :].rearrange("p a b -> p (a b)"),
                            ps1[:, :nw2 * P])
                        pts.append(Pt)

                    # L2: one CONTIGUOUS matmul burst per dst tile (PSUM
                    # accumulation groups must not interleave within a bank)
                    # all TPG dst-tile accumulators packed into one bank
                    l2t = psL2.tile([P, TPG, OF], f32, tag="l2",
                                    name=f"l2ps_{g}")
                    for t in tiles_g:
                        my_mms = [i for i in range(mm0, mm0 + nmm)
                                  if int(pl.mm_t[i]) == t]
                        reg = l2t[:, t - g * TPG, :]
                        # bias: out_tile += deg_w (x) bias   (rank-1)
                        nc.tensor.matmul(
                            reg,
                            lhsT=dw2[:, t * P:(t + 1) * P],
                            rhs=b2[:],
                            start=True,
                            stop=not my_mms,
                        )
                        for q, i in enumerate(my_mms):
                            j = int(pl.mm_j[i])
                            nc.tensor.matmul(
                                reg,
                                lhsT=s2t[:, (i - mm0) * P:(i - mm0 + 1) * P],
                                rhs=pts[j // 4][:, j % 4, :],
                                start=False,
                                stop=(q == len(my_mms) - 1),
                            )

                    ntg = len(tiles_g)
                    t0 = tiles_g[0]
                    ot = pout.tile([P, TPG, OF], f32, tag="out")
                    nc.vector.tensor_copy(
                        ot[:, :ntg, :].rearrange("p a b -> p (a b)"),
                        l2t[:, :ntg, :].rearrange("p a b -> p (a b)"))
                    nc.sync.dma_start(
                        out_d[t0 * P:(t0 + ntg) * P, :].rearrange(
                            "(t p) f -> p t f", p=P),
                        ot[:, :ntg, :])

    nc.compile()
    return nc


# ---------------------------------------------------------------------------
# Entry point
# ---------------------------------------------------------------------------

def kernel(x, edge_src, edge_dst, edge_vals, weight, bias,
           _want_trace=False, _n_cores=None):
    x = np.asarray(x)
    edge_src = np.asarray(edge_src)
    edge_dst = np.asarray(edge_dst)
    edge_vals = np.asarray(edge_vals)
    weight = np.asarray(weight)
    bias = np.asarray(bias)

    pl = build_plan(x, edge_src, edge_dst, edge_vals, weight, bias)
    nc = build_bass(pl)

    from concourse.bass_utils import run_bass_kernel_spmd

    ncores = N_CORES if _n_cores is None else _n_cores
    in_maps = []
    for ci in range(ncores):
        in_maps.append({
            "xt": pl.xT,
            "w": pl.W,
            "bvec": pl.bvec,
            "degw": np.ascontiguousarray(pl.degw[ci]),
            "idx": np.ascontiguousarray(pl.IDX[ci]),
            "bmat": np.ascontiguousarray(pl.Bf[ci]),
            "s2": np.ascontiguousarray(pl.S2f[ci]),
        })
    res = run_bass_kernel_spmd(nc, in_maps, core_ids=list(range(ncores)),
                               trace=_want_trace)
    outs = [res.results[ci]["out"][:pl.ndst, :] for ci in range(ncores)]
    if ncores < N_CORES:
        outs += [np.zeros((pl.ndst, pl.OUT_F), np.float32)] * (
            N_CORES - ncores)
    full = np.concatenate(outs, axis=0).astype(np.float32)
    if _want_trace:
        kernel._last_results = res
    return full
